# revision 25
# baseline (speedup 1.0000x reference)
"""Trainium2 Bass kernel for nn_AttentionHead (B=4, S=2048, DK=1024).

Single-head attention with input projections:
    qp = q @ wq.T; kp = k @ wk.T; vp = v @ wv.T
    s  = qp @ kp.T / sqrt(dk); attn = softmax(s); out = attn @ vp

Sharding: 8 cores = (batch b in 0..3) x (sequence half h in 0..1).
Each core owns 1024 query rows AND 1024 key/value rows of its batch.
K/V projections are computed once per row (no duplication across the
pair): each core projects only its own 1024 k/v rows, then the pair
exchanges halves with 2-rank AllGathers through HBM bounce buffers
(SPMD-uniform layout: both halves are read back from the AllGather
output in global j-order, so the program is identical on all cores).

Per core (all matmul operands bf16, fp32 PSUM accumulation):
    A: kpT_loc[e,j'] = sum_d wkT[d,e] kT_loc[d,j']   (128 MMs)
       -> bounce -> AllGather(pair) -> kpT[e, 0:2048]
    E: vp_loc[j',e]  = sum_d vT_loc[d,j'] wvT[d,e]   (128 MMs)
       -> bounce -> AllGather(pair) -> vp[j] for all 16 j-tiles
    B: qpT[e,i]  = sum_d wqT[d,e] qT[d,i]            (128 MMs)
    C: sT[j,i]   = sum_e kpT[e,j] qpT[e,i]           (256 MMs)
       eT[j,i]   = exp(sT/32)  (ACT, fused scale, stays in SBUF)
       colsum[i] = sum_j eT[j,i]: partial sums over j on the vector
       engine, then one gpsimd partition_all_reduce per query slice
       (frees the PE of 32 ones-matmuls and 2 PSUM banks -> psmm
       rotates 8 banks)
    F: outT[e,i] = (sum_j vp[j,e] eT[j,i]) * (1/colsum[i])  (256 MMs)

DMA-descriptor generation is the hidden serial resource: each
dma_start costs ~0.6us of descriptor generation on its issuing
engine's sequencer, strictly in program order. Input streams use
[128,1024] chunks (2KB per partition row, half the descriptor count
of [128,512]). Ring assignment:
  sync ring  (no compute): wk, wv, vs, wq, q, then both collective
      readbacks -- it can safely stall on the collective-done sems.
      kp readback is front-loaded ([128,256] quarter wave first) so
      C's first j-tiles unblock right after the collective lands and
      delivery outpaces C's ~3.4us/j-tile consumption.
  scalar ring (runs C's activations): k stream, A/E bounce writes,
      F output stores -- all strictly before/after its activation
      window. Readbacks must NOT go here: their collective-done waits
      would queue C's activations behind them and deadlock the PSUM
      rotation (~44us stall, measured).
Both readbacks are emitted after phase B so the vp collective's
auto-generated DMA-lane fence is not dragged behind the kp readback
descriptors in the schedule.

Phase-F accumulators come from the same rotating PSUM pool as the
earlier phases (a fresh pool would wait on the colsum/reciprocal
chain before its first bank frees and stall the PE). The last output
tile runs as two 256-wide column chains so its mul+store overlaps
the final matmuls instead of trailing them.

918 matmuls/core; steady-state issue interval is the full-rate 216ns
(512 cols @ ~2.4GHz; LDWEIGHTS prefetch fully hidden). Measured
end-to-end relative error vs the fp32 reference: ~6e-3 (bf16
quantization). HW exec ~219us at nominal clock (baseline 248us).
"""

import numpy as np

_B, _S, _DK = 4, 2048, 1024
_HALF = _S // 2
_N_CORES = 8
_P = 128
_PAIRS = [[0, 1], [2, 3], [4, 5], [6, 7]]

_CACHE = {}


def _emit(tc, qT, kTh, vTh, wqT, wkT, wvT, outT, cc, DK, S, HALF, mm_dt):
    import concourse.bass as bass
    from concourse import bass_isa, mybir

    nc = tc.nc
    ts = bass.ts
    P = _P
    NF = 512
    KH = S // 2            # local key/value rows
    DT = DK // P           # contraction tiles (d)
    ET = DK // P           # output-feature tiles (e)
    JT = S // P            # global key tiles (j)
    ISL = HALF // NF       # query slices (i)
    JSLH = KH // NF        # local key slices
    ESL = DK // NF         # feature slices
    JGN = KH // NF         # local vT chunk groups
    JPG = NF // P          # j-tiles per vT chunk
    NORM = 1.0 / float(np.sqrt(DK))
    f32 = mybir.dt.float32
    AF = mybir.ActivationFunctionType
    kp_in, kp_out, vp_in, vp_out = cc

    _cms = {}

    def opn(**kw):
        cm = tc.tile_pool(**kw)
        pool = cm.__enter__()
        _cms[id(pool)] = cm
        return pool

    def cls(*pools):
        for pool in pools:
            _cms.pop(id(pool)).__exit__(None, None, None)

    # ---------------- pools ----------------
    # LEFT stack: misc | x (stream rotation) | kpT | vp | qpT
    # RIGHT stack: stage | cst (bounce staging) | weights | eT
    misc = opn(name="misc", bufs=1, side="left")
    xp = opn(name="xp", bufs=1, side="left")
    stage = opn(name="stage", bufs=4, side="right")
    cstp = opn(name="cstp", bufs=1, side="right")
    wp = opn(name="wp", bufs=1, side="right")
    psmm = opn(name="psmm", bufs=8, space="PSUM")

    ones = misc.tile([P, P], mm_dt, tag="ones")
    nc.vector.memset(ones[:], 1.0)
    recip = misc.tile([P, HALF], f32, tag="recip")
    cs_acc = [
        misc.tile([P, NF], f32, tag=f"csa{i}", name=f"csa{i}") for i in range(ISL)
    ]
    cs_red = [
        misc.tile([P, NF], f32, tag=f"csr{i}", name=f"csr{i}") for i in range(ISL)
    ]

    # input stream tiles are all dedicated (no slot rotation, no waits):
    # k and vs in the x pool as full [P, KH] rows, q in the spare wv slots.
    def cst_tile(nm):
        return cstp.tile([P, NF], mm_dt, tag="cst", bufs=12, name=nm)

    # big static tiles allocated up front so the k stream pool (dead
    # after A) is the top of the left stack and can be released early
    kp_pool = opn(name="kpp", bufs=1, side="left")
    kpT = [kp_pool.tile([P, S], mm_dt, tag=f"kp{e}", name=f"kp{e}") for e in range(ET)]
    vp_pool = opn(name="vpp", bufs=1, side="left")
    vp = [vp_pool.tile([P, DK], mm_dt, tag=f"vp{j}", name=f"vp{j}") for j in range(JT)]
    kxp = opn(name="kxp", bufs=1, side="left")

    # ---------------- PE warm-up while first DMAs land ----------------
    warm_ps = psmm.tile([P, P], f32, tag="mm", name="warm_ps")
    for _ in range(24):
        nc.tensor.matmul(warm_ps[:], ones[:], ones[:], start=True, stop=True)

    # All six inputs arrive host-rearranged into (128, 8192) partition-
    # major layouts (16KB per partition row), so each stream is one or
    # two large contiguous descriptors instead of 8-16: input desc-gen
    # drops from ~30us to ~6us, the first A chain's tiles arrive as one
    # block, and the input-transfer completion (which bounds the kp
    # collective's fence) moves ~8us earlier.
    #   k/q:  free order [half, d, 512]   (half = js / isl)
    #   wk:   free order [h, d, 512]      (h = e//4)
    #   wv/wq/vs: free order [d, 1024]
    EPC = NF // P
    HB = DT * NF  # 4096: one half-block of the [half, d, 512] layouts

    # critical first-chain streams (k, wk h0) go at [128,1024] grain so
    # data streams in d-pair by d-pair instead of one late block; bulk
    # streams (wv, vs, wq) stay as two big descs each.
    DP = 2 * NF
    k_all = kxp.tile([P, DT * KH], mm_dt, tag="k", name="k_all")
    for o in range(0, 2 * HB, DP):
        nc.scalar.dma_start(k_all[:, o : o + DP], kTh[:, o : o + DP])
    wk_all = wp.tile([P, DT * DK], mm_dt, tag="wk", name="wk_all")
    for o in range(0, 2 * HB, DP):
        nc.sync.dma_start(wk_all[:, o : o + DP], wkT[:, o : o + DP])
    wv_all = wp.tile([P, DT * DK], mm_dt, tag="wv", name="wv_all")
    nc.sync.dma_start(wv_all[:, 0:HB], wvT[:, 0:HB])
    nc.sync.dma_start(wv_all[:, HB : 2 * HB], wvT[:, HB : 2 * HB])
    vs_all = xp.tile([P, DT * KH], mm_dt, tag="vs", name="vs_all")
    nc.sync.dma_start(vs_all[:, 0:HB], vTh[:, 0:HB])
    nc.sync.dma_start(vs_all[:, HB : 2 * HB], vTh[:, HB : 2 * HB])

    def wk_slice(d, e):
        o = (e // EPC) * HB + d * NF + (e % EPC) * P
        return wk_all[:, o : o + P]

    # ---------------- phase A: local kpT half -> bounce -> AllGather ----
    for js in range(JSLH):
        for e in range(ET):
            ps = psmm.tile([P, NF], f32, tag="mm")
            for d in range(DT):
                nc.tensor.matmul(
                    ps[:],
                    wk_slice(d, e),
                    k_all[:, js * HB + d * NF : js * HB + (d + 1) * NF],
                    start=(d == 0),
                    stop=(d == DT - 1),
                )
            st = cst_tile(f"kpb{js}_{e}")
            nc.vector.tensor_copy(st[:], ps[:])
            nc.scalar.dma_start(kp_in[ts(e, P), ts(js, NF)], st[:])
    nc.gpsimd.collective_compute(
        "AllGather",
        mybir.AluOpType.bypass,
        replica_groups=_PAIRS,
        ins=[kp_in[:, :]],
        outs=[kp_out[:, :]],
    )

    # wq + q loads ride the sync ring behind the vs stream
    wq_all = wp.tile([P, DT * DK], mm_dt, tag="wq", name="wq_all")
    nc.sync.dma_start(wq_all[:, 0:HB], wqT[:, 0:HB])
    nc.sync.dma_start(wq_all[:, HB : 2 * HB], wqT[:, HB : 2 * HB])
    q_all = wp.tile([P, DT * DK], mm_dt, tag="q", name="q_all")
    for o in range(0, 2 * HB, DP):
        nc.sync.dma_start(q_all[:, o : o + DP], qT[:, o : o + DP])

    # ---------------- phase E: local vp half -> bounce -> AllGather ----
    for g in range(JGN):
        for jin in range(JPG):
            for es in range(ESL):
                ps = psmm.tile([P, NF], f32, tag="mm")
                for d in range(DT):
                    nc.tensor.matmul(
                        ps[:],
                        vs_all[
                            :,
                            d * DK + (g * JPG + jin) * P : d * DK
                            + (g * JPG + jin + 1) * P,
                        ],
                        wv_all[:, d * DK + es * NF : d * DK + (es + 1) * NF],
                        start=(d == 0),
                        stop=(d == DT - 1),
                    )
                st = cst_tile(f"vpb{g}_{jin}_{es}")
                nc.vector.tensor_copy(st[:], ps[:])
                nc.scalar.dma_start(
                    vp_in[ts(g * JPG + jin, P), ts(es, NF)], st[:]
                )
    nc.gpsimd.collective_compute(
        "AllGather",
        mybir.AluOpType.bypass,
        replica_groups=_PAIRS,
        ins=[vp_in[:, :]],
        outs=[vp_out[:, :]],
    )

    # ---------------- phase B: qpT = (q @ wq.T).T ----------------
    # k stream tiles are dead after A; free their SBUF so qpT fits
    cls(kxp)
    qp_pool = opn(name="qpp", bufs=1, side="left")
    qpT = [
        qp_pool.tile([P, HALF], mm_dt, tag=f"qp{e}", name=f"qp{e}") for e in range(ET)
    ]
    for isl in range(ISL):
        for e in range(ET):
            ps = psmm.tile([P, NF], f32, tag="mm")
            for d in range(DT):
                nc.tensor.matmul(
                    ps[:],
                    wq_all[:, d * DK + e * P : d * DK + (e + 1) * P],
                    q_all[:, isl * HB + d * NF : isl * HB + (d + 1) * NF],
                    start=(d == 0),
                    stop=(d == DT - 1),
                )
            nc.vector.tensor_copy(qpT[e][:, ts(isl, NF)], ps[:])

    # Collective readbacks, both emitted after B so the vp collective's
    # auto-generated DMA barrier isn't dragged behind them in the
    # schedule. Sync ring (no compute shares that sequencer; it can
    # safely stall on the collective-done semaphore). kp first: its
    # collective lands earlier and C consumes it first. kp is
    # front-loaded as a [128,256] quarter wave (unblocks j0-1 right
    # after the collective lands), then [128,768] remainders, then
    # hr=1 full rows; desc-gen then outpaces C's consumption.
    QW = 256
    for e in range(ET):
        nc.sync.dma_start(kpT[e][:, 0:QW], kp_out[ts(e, P), 0:QW])
    for e in range(ET):
        nc.sync.dma_start(kpT[e][:, QW:KH], kp_out[ts(e, P), QW:KH])
    for e in range(ET):
        nc.sync.dma_start(kpT[e][:, KH : 2 * KH], kp_out[ts(ET + e, P), :])
    for hr in range(2):
        for jl in range(JT // 2):
            nc.sync.dma_start(
                vp[hr * (JT // 2) + jl][:, :], vp_out[ts(hr * (JT // 2) + jl, P), :]
            )

    # weights are dead after B; free their SBUF so eT can live there
    cls(wp)

    # ---------------- phase C: sT -> exp -> eT (SBUF) + trailing colsum ----
    et_pool = opn(name="etp", bufs=1, side="right")
    eT = [et_pool.tile([P, HALF], mm_dt, tag=f"et{j}", name=f"et{j}") for j in range(JT)]
    for j in range(JT):
        for isl in range(ISL):
            ps = psmm.tile([P, NF], f32, tag="mm")
            for e in range(ET):
                nc.tensor.matmul(
                    ps[:],
                    kpT[e][:, ts(j, P)],
                    qpT[e][:, ts(isl, NF)],
                    start=(e == 0),
                    stop=(e == ET - 1),
                )
            nc.scalar.activation(eT[j][:, ts(isl, NF)], ps[:], AF.Exp, scale=NORM)
            # colsum rides the vector engine (partial sums over j) plus a
            # gpsimd cross-partition reduce, freeing the PE of 32
            # ones-matmuls
            if j == 0:
                nc.vector.tensor_copy(cs_acc[isl][:], eT[0][:, ts(isl, NF)])
            else:
                nc.vector.tensor_tensor(
                    cs_acc[isl][:],
                    cs_acc[isl][:],
                    eT[j][:, ts(isl, NF)],
                    op=mybir.AluOpType.add,
                )
            if j == JT - 1:
                nc.gpsimd.partition_all_reduce(
                    cs_red[isl][:],
                    cs_acc[isl][:],
                    channels=P,
                    reduce_op=bass_isa.ReduceOp.add,
                )
                nc.vector.reciprocal(recip[:, ts(isl, NF)], cs_red[isl][:])

    # ---------------- phase F: outT = (eT.T @ vp).T * recip ----------------
    # Accumulators come from the same rotating psmm pool (a fresh PSUM pool
    # would wait on the colsum/reciprocal chain before its first bank frees).
    # The last (isl,e) tile runs as two 256-wide column chains so its
    # mul+store overlaps the final matmuls instead of trailing them.
    HW = NF // 2
    for isl in range(ISL):
        for e in range(ET):
            if isl == ISL - 1 and e == ET - 1:
                for half in range(2):
                    pfh = psmm.tile([P, NF], f32, tag="mm", name=f"pfl{half}")
                    for j in range(JT):
                        nc.tensor.matmul(
                            pfh[:, 0:HW],
                            vp[j][:, ts(e, P)],
                            eT[j][:, isl * NF + half * HW : isl * NF + (half + 1) * HW],
                            start=(j == 0),
                            stop=(j == JT - 1),
                        )
                    oth = stage.tile([P, HW], f32, tag="ostl", bufs=2)
                    nc.vector.tensor_mul(
                        oth[:],
                        pfh[:, 0:HW],
                        recip[:, isl * NF + half * HW : isl * NF + (half + 1) * HW],
                    )
                    nc.scalar.dma_start(
                        outT[
                            ts(e, P),
                            isl * NF + half * HW : isl * NF + (half + 1) * HW,
                        ],
                        oth[:],
                    )
                continue
            pft = psmm.tile([P, NF], f32, tag="mm", name=f"pf{e}_{isl}")
            for j in range(JT):
                nc.tensor.matmul(
                    pft[:],
                    vp[j][:, ts(e, P)],
                    eT[j][:, ts(isl, NF)],
                    start=(j == 0),
                    stop=(j == JT - 1),
                )
            ot = stage.tile([P, NF], f32, tag="ost")
            nc.vector.tensor_mul(ot[:], pft[:], recip[:, ts(isl, NF)])
            nc.scalar.dma_start(outT[ts(e, P), ts(isl, NF)], ot[:])
    cls(qp_pool, vp_pool, kp_pool, xp, misc)
    cls(et_pool, cstp, stage)
    cls(psmm)


def build_program(DK=_DK, S=_S, HALF=_HALF, mm_dtype="bfloat16"):
    """Build + compile the per-core Bass program. Returns the Bacc object."""
    import concourse.tile as tile
    from concourse import bacc, mybir

    f32 = mybir.dt.float32
    mm_dt = getattr(mybir.dt, mm_dtype)
    KH = S // 2
    NF = 512

    nc = bacc.Bacc(
        "TRN2",
        target_bir_lowering=False,
        debug=False,
        enable_asserts=False,
        num_devices=_N_CORES,
    )
    # all inputs host-rearranged to (128, 8192) partition-major layouts
    BIG = (DK // 128) * DK  # 8192
    qT = nc.dram_tensor("qt", (128, BIG), mm_dt, kind="ExternalInput").ap()
    kTh = nc.dram_tensor("kt", (128, BIG), mm_dt, kind="ExternalInput").ap()
    vTh = nc.dram_tensor("vt", (128, BIG), mm_dt, kind="ExternalInput").ap()
    wqT = nc.dram_tensor("wqt", (128, BIG), mm_dt, kind="ExternalInput").ap()
    wkT = nc.dram_tensor("wkt", (128, BIG), mm_dt, kind="ExternalInput").ap()
    wvT = nc.dram_tensor("wvt", (128, BIG), mm_dt, kind="ExternalInput").ap()
    outT = nc.dram_tensor("outt", (DK, HALF), f32, kind="ExternalOutput").ap()

    # HBM bounce buffers for the pair AllGathers (one per tensor: each
    # collective carries a ~25us firmware latency floor, so fewer is faster)
    kp_in = nc.dram_tensor("kp_in", (DK, KH), mm_dt, kind="Internal").ap()
    kp_out = nc.dram_tensor("kp_out", (2 * DK, KH), mm_dt, kind="Internal").ap()
    vp_in = nc.dram_tensor("vp_in", (KH, DK), mm_dt, kind="Internal").ap()
    vp_out = nc.dram_tensor("vp_out", (2 * KH, DK), mm_dt, kind="Internal").ap()

    with tile.TileContext(nc) as tc:
        _emit(
            tc,
            qT,
            kTh,
            vTh,
            wqT,
            wkT,
            wvT,
            outT,
            (kp_in, kp_out, vp_in, vp_out),
            DK,
            S,
            HALF,
            mm_dt,
        )
    nc.compile()
    return nc


def _half_major(aT):
    """(1024, 1024) -> (128, 8192), free order [half, d, 512]."""
    return np.ascontiguousarray(
        aT.reshape(8, 128, 2, 512).transpose(1, 2, 0, 3).reshape(128, 8192)
    )


def _d_major(aT):
    """(1024, 1024) -> (128, 8192), free order [d, 1024]."""
    return np.ascontiguousarray(
        aT.reshape(8, 128, 1024).transpose(1, 0, 2).reshape(128, 8192)
    )


def _in_maps(q, k, v, wq, wk, wv):
    """Shard full inputs into 8 per-core input maps.

    Host-side transposes plus a partition-major rearrange to (128, 8192)
    so every device-side input stream is 1-2 large contiguous DMA
    descriptors (16KB per partition row)."""
    import ml_dtypes

    bf16 = ml_dtypes.bfloat16
    wqT = _d_major(np.ascontiguousarray(wq.T).astype(bf16))
    wkT = _half_major(np.ascontiguousarray(wk.T).astype(bf16))
    wvT = _d_major(np.ascontiguousarray(wv.T).astype(bf16))
    maps = []
    for c in range(_N_CORES):
        b, h = divmod(c, 2)
        sl = slice(h * _HALF, (h + 1) * _HALF)
        maps.append(
            {
                "qt": _half_major(np.ascontiguousarray(q[b, sl, :].T).astype(bf16)),
                "kt": _half_major(np.ascontiguousarray(k[b, sl, :].T).astype(bf16)),
                "vt": _d_major(np.ascontiguousarray(v[b, sl, :].T).astype(bf16)),
                "wqt": wqT,
                "wkt": wkT,
                "wvt": wvT,
            }
        )
    return maps


def kernel(q, k, v, wq, wk, wv):
    from concourse.bass_utils import run_bass_kernel_spmd

    q = np.asarray(q, np.float32)
    k = np.asarray(k, np.float32)
    v = np.asarray(v, np.float32)
    wq = np.asarray(wq, np.float32)
    wk = np.asarray(wk, np.float32)
    wv = np.asarray(wv, np.float32)

    if "nc" not in _CACHE:
        _CACHE["nc"] = build_program()
    nc = _CACHE["nc"]

    res = run_bass_kernel_spmd(
        nc, _in_maps(q, k, v, wq, wk, wv), core_ids=list(range(_N_CORES))
    )

    out = np.empty((_B, _S, _DK), np.float32)
    for c in range(_N_CORES):
        b, h = divmod(c, 2)
        out[b, h * _HALF : (h + 1) * _HALF, :] = res.results[c]["outt"].T
    return out


# revision 26
# speedup vs baseline: 1.0008x; 1.0008x over previous
"""Trainium2 Bass kernel for nn_AttentionHead (B=4, S=2048, DK=1024).

Single-head attention with input projections:
    qp = q @ wq.T; kp = k @ wk.T; vp = v @ wv.T
    s  = qp @ kp.T / sqrt(dk); attn = softmax(s); out = attn @ vp

Sharding: 8 cores = (batch b in 0..3) x (sequence half h in 0..1).
Each core owns 1024 query rows AND 1024 key/value rows of its batch.
K/V projections are computed once per row (no duplication across the
pair): each core projects only its own 1024 k/v rows, then the pair
exchanges halves with 2-rank AllGathers through HBM bounce buffers
(SPMD-uniform layout: both halves are read back from the AllGather
output in global j-order, so the program is identical on all cores).

Per core (all matmul operands bf16, fp32 PSUM accumulation):
    A: kpT_loc[e,j'] = sum_d wkT[d,e] kT_loc[d,j']   (128 MMs)
       -> bounce -> AllGather(pair) -> kpT[e, 0:2048]
    E: vp_loc[j',e]  = sum_d vT_loc[d,j'] wvT[d,e]   (128 MMs)
       -> bounce -> AllGather(pair) -> vp[j] for all 16 j-tiles
    B: qpT[e,i]  = sum_d wqT[d,e] qT[d,i]            (128 MMs)
    C: sT[j,i]   = sum_e kpT[e,j] qpT[e,i]           (256 MMs)
       eT[j,i]   = exp(sT/32)  (ACT, fused scale, stays in SBUF)
       colsum[i] = sum_j eT[j,i]: partial sums over j on the vector
       engine, then one gpsimd partition_all_reduce per query slice
       (frees the PE of 32 ones-matmuls and 2 PSUM banks -> psmm
       rotates 8 banks)
    F: outT[e,i] = (sum_j vp[j,e] eT[j,i]) * (1/colsum[i])  (256 MMs)

DMA-descriptor generation is the hidden serial resource: each
dma_start costs ~0.6us of descriptor generation on its issuing
engine's sequencer, strictly in program order. Input streams use
[128,1024] chunks (2KB per partition row, half the descriptor count
of [128,512]). Ring assignment:
  sync ring  (no compute): wk, wv, vs, wq, q, then both collective
      readbacks -- it can safely stall on the collective-done sems.
      kp readback is front-loaded ([128,256] quarter wave first) so
      C's first j-tiles unblock right after the collective lands and
      delivery outpaces C's ~3.4us/j-tile consumption.
  scalar ring (runs C's activations): k stream, A/E bounce writes,
      F output stores -- all strictly before/after its activation
      window. Readbacks must NOT go here: their collective-done waits
      would queue C's activations behind them and deadlock the PSUM
      rotation (~44us stall, measured).
Both readbacks are emitted after phase B so the vp collective's
auto-generated DMA-lane fence is not dragged behind the kp readback
descriptors in the schedule.

Phase-F accumulators come from the same rotating PSUM pool as the
earlier phases (a fresh pool would wait on the colsum/reciprocal
chain before its first bank frees and stall the PE). The last output
tile runs as two 256-wide column chains so its mul+store overlaps
the final matmuls instead of trailing them.

918 matmuls/core; steady-state issue interval is the full-rate 216ns
(512 cols @ ~2.4GHz; LDWEIGHTS prefetch fully hidden). Measured
end-to-end relative error vs the fp32 reference: ~6e-3 (bf16
quantization). HW exec ~219us at nominal clock (baseline 248us).
"""

import numpy as np

_B, _S, _DK = 4, 2048, 1024
_HALF = _S // 2
_N_CORES = 8
_P = 128
_PAIRS = [[0, 1], [2, 3], [4, 5], [6, 7]]

_CACHE = {}


def _emit(tc, qT, kTh, vTh, wqT, wkT, wvT, outT, cc, DK, S, HALF, mm_dt):
    import concourse.bass as bass
    from concourse import bass_isa, mybir

    nc = tc.nc
    ts = bass.ts
    P = _P
    NF = 512
    KH = S // 2            # local key/value rows
    DT = DK // P           # contraction tiles (d)
    ET = DK // P           # output-feature tiles (e)
    JT = S // P            # global key tiles (j)
    ISL = HALF // NF       # query slices (i)
    JSLH = KH // NF        # local key slices
    ESL = DK // NF         # feature slices
    JGN = KH // NF         # local vT chunk groups
    JPG = NF // P          # j-tiles per vT chunk
    NORM = 1.0 / float(np.sqrt(DK))
    f32 = mybir.dt.float32
    AF = mybir.ActivationFunctionType
    kp_in, kp_out, vp_in, vp_out = cc

    _cms = {}

    def opn(**kw):
        cm = tc.tile_pool(**kw)
        pool = cm.__enter__()
        _cms[id(pool)] = cm
        return pool

    def cls(*pools):
        for pool in pools:
            _cms.pop(id(pool)).__exit__(None, None, None)

    # ---------------- pools ----------------
    # LEFT stack: misc | x (stream rotation) | kpT | vp | qpT
    # RIGHT stack: stage | cst (bounce staging) | weights | eT
    misc = opn(name="misc", bufs=1, side="left")
    xp = opn(name="xp", bufs=1, side="left")
    stage = opn(name="stage", bufs=4, side="right")
    cstp = opn(name="cstp", bufs=1, side="right")
    wp = opn(name="wp", bufs=1, side="right")
    psmm = opn(name="psmm", bufs=8, space="PSUM")

    ones = misc.tile([P, P], mm_dt, tag="ones")
    nc.vector.memset(ones[:], 1.0)
    recip = misc.tile([P, HALF], f32, tag="recip")
    cs_acc = [
        misc.tile([P, NF], f32, tag=f"csa{i}", name=f"csa{i}") for i in range(ISL)
    ]
    cs_red = [
        misc.tile([P, NF], f32, tag=f"csr{i}", name=f"csr{i}") for i in range(ISL)
    ]

    # input stream tiles are all dedicated (no slot rotation, no waits):
    # k and vs in the x pool as full [P, KH] rows, q in the spare wv slots.
    def cst_tile(nm):
        return cstp.tile([P, NF], mm_dt, tag="cst", bufs=12, name=nm)

    # big static tiles allocated up front so the k stream pool (dead
    # after A) is the top of the left stack and can be released early
    kp_pool = opn(name="kpp", bufs=1, side="left")
    kpT = [kp_pool.tile([P, S], mm_dt, tag=f"kp{e}", name=f"kp{e}") for e in range(ET)]
    vp_pool = opn(name="vpp", bufs=1, side="left")
    vp = [vp_pool.tile([P, DK], mm_dt, tag=f"vp{j}", name=f"vp{j}") for j in range(JT)]
    kxp = opn(name="kxp", bufs=1, side="left")

    # ---------------- PE warm-up while first DMAs land ----------------
    warm_ps = psmm.tile([P, P], f32, tag="mm", name="warm_ps")
    for _ in range(24):
        nc.tensor.matmul(warm_ps[:], ones[:], ones[:], start=True, stop=True)

    # All six inputs arrive host-rearranged into (128, 8192) partition-
    # major layouts (16KB per partition row), so each stream is one or
    # two large contiguous descriptors instead of 8-16: input desc-gen
    # drops from ~30us to ~6us, the first A chain's tiles arrive as one
    # block, and the input-transfer completion (which bounds the kp
    # collective's fence) moves ~8us earlier.
    #   k/q:  free order [half, d, 512]   (half = js / isl)
    #   wk:   free order [h, d, 512]      (h = e//4)
    #   wv/wq/vs: free order [d, 1024]
    EPC = NF // P
    HB = DT * NF  # 4096: one half-block of the [half, d, 512] layouts

    # critical first-chain streams (k, wk h0) go at [128,1024] grain so
    # data streams in d-pair by d-pair instead of one late block; bulk
    # streams (wv, vs, wq) stay as two big descs each.
    DP = 2 * NF
    k_all = kxp.tile([P, DT * KH], mm_dt, tag="k", name="k_all")
    for o in range(0, 2 * HB, DP):
        nc.scalar.dma_start(k_all[:, o : o + DP], kTh[:, o : o + DP])
    wk_all = wp.tile([P, DT * DK], mm_dt, tag="wk", name="wk_all")
    for o in range(0, 2 * HB, DP):
        nc.sync.dma_start(wk_all[:, o : o + DP], wkT[:, o : o + DP])
    wv_all = wp.tile([P, DT * DK], mm_dt, tag="wv", name="wv_all")
    nc.sync.dma_start(wv_all[:, 0:HB], wvT[:, 0:HB])
    nc.sync.dma_start(wv_all[:, HB : 2 * HB], wvT[:, HB : 2 * HB])
    vs_all = xp.tile([P, DT * KH], mm_dt, tag="vs", name="vs_all")
    nc.sync.dma_start(vs_all[:, 0:HB], vTh[:, 0:HB])
    nc.sync.dma_start(vs_all[:, HB : 2 * HB], vTh[:, HB : 2 * HB])

    def wk_slice(d, e):
        o = (e // EPC) * HB + d * NF + (e % EPC) * P
        return wk_all[:, o : o + P]

    # ---------------- phase A: local kpT half -> bounce -> AllGather ----
    for js in range(JSLH):
        for e in range(ET):
            ps = psmm.tile([P, NF], f32, tag="mm")
            for d in range(DT):
                nc.tensor.matmul(
                    ps[:],
                    wk_slice(d, e),
                    k_all[:, js * HB + d * NF : js * HB + (d + 1) * NF],
                    start=(d == 0),
                    stop=(d == DT - 1),
                )
            st = cst_tile(f"kpb{js}_{e}")
            nc.vector.tensor_copy(st[:], ps[:])
            nc.scalar.dma_start(kp_in[ts(e, P), ts(js, NF)], st[:])
    nc.gpsimd.collective_compute(
        "AllGather",
        mybir.AluOpType.bypass,
        replica_groups=_PAIRS,
        ins=[kp_in[:, :]],
        outs=[kp_out[:, :]],
    )

    # wq + q loads ride the sync ring behind the vs stream
    wq_all = wp.tile([P, DT * DK], mm_dt, tag="wq", name="wq_all")
    nc.sync.dma_start(wq_all[:, 0:HB], wqT[:, 0:HB])
    nc.sync.dma_start(wq_all[:, HB : 2 * HB], wqT[:, HB : 2 * HB])
    q_all = wp.tile([P, DT * DK], mm_dt, tag="q", name="q_all")
    for o in range(0, 2 * HB, DP):
        nc.sync.dma_start(q_all[:, o : o + DP], qT[:, o : o + DP])

    # ---------------- phase E: local vp half -> bounce -> AllGather ----
    for g in range(JGN):
        for jin in range(JPG):
            for es in range(ESL):
                ps = psmm.tile([P, NF], f32, tag="mm")
                for d in range(DT):
                    nc.tensor.matmul(
                        ps[:],
                        vs_all[
                            :,
                            d * DK + (g * JPG + jin) * P : d * DK
                            + (g * JPG + jin + 1) * P,
                        ],
                        wv_all[:, d * DK + es * NF : d * DK + (es + 1) * NF],
                        start=(d == 0),
                        stop=(d == DT - 1),
                    )
                st = cst_tile(f"vpb{g}_{jin}_{es}")
                nc.vector.tensor_copy(st[:], ps[:])
                nc.scalar.dma_start(
                    vp_in[ts(g * JPG + jin, P), ts(es, NF)], st[:]
                )
    nc.gpsimd.collective_compute(
        "AllGather",
        mybir.AluOpType.bypass,
        replica_groups=_PAIRS,
        ins=[vp_in[:, :]],
        outs=[vp_out[:, :]],
    )

    # ---------------- phase B: qpT = (q @ wq.T).T ----------------
    # k stream tiles are dead after A; free their SBUF so qpT fits
    cls(kxp)
    qp_pool = opn(name="qpp", bufs=1, side="left")
    qpT = [
        qp_pool.tile([P, HALF], mm_dt, tag=f"qp{e}", name=f"qp{e}") for e in range(ET)
    ]
    for isl in range(ISL):
        for e in range(ET):
            ps = psmm.tile([P, NF], f32, tag="mm")
            for d in range(DT):
                nc.tensor.matmul(
                    ps[:],
                    wq_all[:, d * DK + e * P : d * DK + (e + 1) * P],
                    q_all[:, isl * HB + d * NF : isl * HB + (d + 1) * NF],
                    start=(d == 0),
                    stop=(d == DT - 1),
                )
            nc.vector.tensor_copy(qpT[e][:, ts(isl, NF)], ps[:])

    # Collective readbacks, both emitted after B so the vp collective's
    # auto-generated DMA barrier isn't dragged behind them in the
    # schedule. Sync ring (no compute shares that sequencer; it can
    # safely stall on the collective-done semaphore). kp first: its
    # collective lands earlier and C consumes it first. kp is
    # front-loaded as a [128,256] quarter wave (unblocks j0-1 right
    # after the collective lands), then [128,768] remainders, then
    # hr=1 full rows; desc-gen then outpaces C's consumption.
    QW = 256
    for e in range(4):
        nc.sync.dma_start(kpT[e][:, 0:QW], kp_out[ts(e, P), 0:QW])
    for e in range(4, ET):
        nc.scalar.dma_start(kpT[e][:, 0:QW], kp_out[ts(e, P), 0:QW])
    for e in range(ET):
        nc.sync.dma_start(kpT[e][:, QW:KH], kp_out[ts(e, P), QW:KH])
    for e in range(ET):
        nc.sync.dma_start(kpT[e][:, KH : 2 * KH], kp_out[ts(ET + e, P), :])
    for hr in range(2):
        for jl in range(JT // 2):
            nc.sync.dma_start(
                vp[hr * (JT // 2) + jl][:, :], vp_out[ts(hr * (JT // 2) + jl, P), :]
            )

    # weights are dead after B; free their SBUF so eT can live there
    cls(wp)

    # ---------------- phase C: sT -> exp -> eT (SBUF) + trailing colsum ----
    et_pool = opn(name="etp", bufs=1, side="right")
    eT = [et_pool.tile([P, HALF], mm_dt, tag=f"et{j}", name=f"et{j}") for j in range(JT)]
    for j in range(JT):
        for isl in range(ISL):
            ps = psmm.tile([P, NF], f32, tag="mm")
            for e in range(ET):
                nc.tensor.matmul(
                    ps[:],
                    kpT[e][:, ts(j, P)],
                    qpT[e][:, ts(isl, NF)],
                    start=(e == 0),
                    stop=(e == ET - 1),
                )
            nc.scalar.activation(eT[j][:, ts(isl, NF)], ps[:], AF.Exp, scale=NORM)
            # colsum rides the vector engine (partial sums over j) plus a
            # gpsimd cross-partition reduce, freeing the PE of 32
            # ones-matmuls
            if j == 0:
                nc.vector.tensor_copy(cs_acc[isl][:], eT[0][:, ts(isl, NF)])
            else:
                nc.vector.tensor_tensor(
                    cs_acc[isl][:],
                    cs_acc[isl][:],
                    eT[j][:, ts(isl, NF)],
                    op=mybir.AluOpType.add,
                )
            if j == JT - 1:
                nc.gpsimd.partition_all_reduce(
                    cs_red[isl][:],
                    cs_acc[isl][:],
                    channels=P,
                    reduce_op=bass_isa.ReduceOp.add,
                )
                nc.vector.reciprocal(recip[:, ts(isl, NF)], cs_red[isl][:])

    # ---------------- phase F: outT = (eT.T @ vp).T * recip ----------------
    # Accumulators come from the same rotating psmm pool (a fresh PSUM pool
    # would wait on the colsum/reciprocal chain before its first bank frees).
    # The last (isl,e) tile runs as two 256-wide column chains so its
    # mul+store overlaps the final matmuls instead of trailing them.
    HW = NF // 2
    for isl in range(ISL):
        for e in range(ET):
            if isl == ISL - 1 and e == ET - 1:
                for half in range(2):
                    pfh = psmm.tile([P, NF], f32, tag="mm", name=f"pfl{half}")
                    for j in range(JT):
                        nc.tensor.matmul(
                            pfh[:, 0:HW],
                            vp[j][:, ts(e, P)],
                            eT[j][:, isl * NF + half * HW : isl * NF + (half + 1) * HW],
                            start=(j == 0),
                            stop=(j == JT - 1),
                        )
                    oth = stage.tile([P, HW], f32, tag="ostl", bufs=2)
                    nc.vector.tensor_mul(
                        oth[:],
                        pfh[:, 0:HW],
                        recip[:, isl * NF + half * HW : isl * NF + (half + 1) * HW],
                    )
                    nc.scalar.dma_start(
                        outT[
                            ts(e, P),
                            isl * NF + half * HW : isl * NF + (half + 1) * HW,
                        ],
                        oth[:],
                    )
                continue
            pft = psmm.tile([P, NF], f32, tag="mm", name=f"pf{e}_{isl}")
            for j in range(JT):
                nc.tensor.matmul(
                    pft[:],
                    vp[j][:, ts(e, P)],
                    eT[j][:, ts(isl, NF)],
                    start=(j == 0),
                    stop=(j == JT - 1),
                )
            ot = stage.tile([P, NF], f32, tag="ost")
            nc.vector.tensor_mul(ot[:], pft[:], recip[:, ts(isl, NF)])
            nc.scalar.dma_start(outT[ts(e, P), ts(isl, NF)], ot[:])
    cls(qp_pool, vp_pool, kp_pool, xp, misc)
    cls(et_pool, cstp, stage)
    cls(psmm)


def build_program(DK=_DK, S=_S, HALF=_HALF, mm_dtype="bfloat16"):
    """Build + compile the per-core Bass program. Returns the Bacc object."""
    import concourse.tile as tile
    from concourse import bacc, mybir

    f32 = mybir.dt.float32
    mm_dt = getattr(mybir.dt, mm_dtype)
    KH = S // 2
    NF = 512

    nc = bacc.Bacc(
        "TRN2",
        target_bir_lowering=False,
        debug=False,
        enable_asserts=False,
        num_devices=_N_CORES,
    )
    # all inputs host-rearranged to (128, 8192) partition-major layouts
    BIG = (DK // 128) * DK  # 8192
    qT = nc.dram_tensor("qt", (128, BIG), mm_dt, kind="ExternalInput").ap()
    kTh = nc.dram_tensor("kt", (128, BIG), mm_dt, kind="ExternalInput").ap()
    vTh = nc.dram_tensor("vt", (128, BIG), mm_dt, kind="ExternalInput").ap()
    wqT = nc.dram_tensor("wqt", (128, BIG), mm_dt, kind="ExternalInput").ap()
    wkT = nc.dram_tensor("wkt", (128, BIG), mm_dt, kind="ExternalInput").ap()
    wvT = nc.dram_tensor("wvt", (128, BIG), mm_dt, kind="ExternalInput").ap()
    outT = nc.dram_tensor("outt", (DK, HALF), f32, kind="ExternalOutput").ap()

    # HBM bounce buffers for the pair AllGathers (one per tensor: each
    # collective carries a ~25us firmware latency floor, so fewer is faster)
    kp_in = nc.dram_tensor("kp_in", (DK, KH), mm_dt, kind="Internal").ap()
    kp_out = nc.dram_tensor("kp_out", (2 * DK, KH), mm_dt, kind="Internal").ap()
    vp_in = nc.dram_tensor("vp_in", (KH, DK), mm_dt, kind="Internal").ap()
    vp_out = nc.dram_tensor("vp_out", (2 * KH, DK), mm_dt, kind="Internal").ap()

    with tile.TileContext(nc) as tc:
        _emit(
            tc,
            qT,
            kTh,
            vTh,
            wqT,
            wkT,
            wvT,
            outT,
            (kp_in, kp_out, vp_in, vp_out),
            DK,
            S,
            HALF,
            mm_dt,
        )
    nc.compile()
    return nc


def _half_major(aT):
    """(1024, 1024) -> (128, 8192), free order [half, d, 512]."""
    return np.ascontiguousarray(
        aT.reshape(8, 128, 2, 512).transpose(1, 2, 0, 3).reshape(128, 8192)
    )


def _d_major(aT):
    """(1024, 1024) -> (128, 8192), free order [d, 1024]."""
    return np.ascontiguousarray(
        aT.reshape(8, 128, 1024).transpose(1, 0, 2).reshape(128, 8192)
    )


def _in_maps(q, k, v, wq, wk, wv):
    """Shard full inputs into 8 per-core input maps.

    Host-side transposes plus a partition-major rearrange to (128, 8192)
    so every device-side input stream is 1-2 large contiguous DMA
    descriptors (16KB per partition row)."""
    import ml_dtypes

    bf16 = ml_dtypes.bfloat16
    wqT = _d_major(np.ascontiguousarray(wq.T).astype(bf16))
    wkT = _half_major(np.ascontiguousarray(wk.T).astype(bf16))
    wvT = _d_major(np.ascontiguousarray(wv.T).astype(bf16))
    maps = []
    for c in range(_N_CORES):
        b, h = divmod(c, 2)
        sl = slice(h * _HALF, (h + 1) * _HALF)
        maps.append(
            {
                "qt": _half_major(np.ascontiguousarray(q[b, sl, :].T).astype(bf16)),
                "kt": _half_major(np.ascontiguousarray(k[b, sl, :].T).astype(bf16)),
                "vt": _d_major(np.ascontiguousarray(v[b, sl, :].T).astype(bf16)),
                "wqt": wqT,
                "wkt": wkT,
                "wvt": wvT,
            }
        )
    return maps


def kernel(q, k, v, wq, wk, wv):
    from concourse.bass_utils import run_bass_kernel_spmd

    q = np.asarray(q, np.float32)
    k = np.asarray(k, np.float32)
    v = np.asarray(v, np.float32)
    wq = np.asarray(wq, np.float32)
    wk = np.asarray(wk, np.float32)
    wv = np.asarray(wv, np.float32)

    if "nc" not in _CACHE:
        _CACHE["nc"] = build_program()
    nc = _CACHE["nc"]

    res = run_bass_kernel_spmd(
        nc, _in_maps(q, k, v, wq, wk, wv), core_ids=list(range(_N_CORES))
    )

    out = np.empty((_B, _S, _DK), np.float32)
    for c in range(_N_CORES):
        b, h = divmod(c, 2)
        out[b, h * _HALF : (h + 1) * _HALF, :] = res.results[c]["outt"].T
    return out


# revision 27
# speedup vs baseline: 1.0230x; 1.0222x over previous
"""Trainium2 Bass kernel for nn_AttentionHead (B=4, S=2048, DK=1024).

Single-head attention with input projections:
    qp = q @ wq.T; kp = k @ wk.T; vp = v @ wv.T
    s  = qp @ kp.T / sqrt(dk); attn = softmax(s); out = attn @ vp

Sharding: 8 cores = (batch b in 0..3) x (sequence half h in 0..1).
Each core owns 1024 query rows AND 1024 key/value rows of its batch.
K/V projections are computed once per row (no duplication across the
pair): each core projects only its own 1024 k/v rows, then the pair
exchanges halves with 2-rank AllGathers through HBM bounce buffers
(SPMD-uniform layout: both halves are read back from the AllGather
output in global j-order, so the program is identical on all cores).

Per core (all matmul operands bf16, fp32 PSUM accumulation):
    A: kpT_loc[e,j'] = sum_d wkT[d,e] kT_loc[d,j']   (128 MMs)
       -> bounce -> AllGather(pair) -> kpT[e, 0:2048]
    E: vp_loc[j',e]  = sum_d vT_loc[d,j'] wvT[d,e]   (128 MMs)
       -> bounce -> AllGather(pair) -> vp[j] for all 16 j-tiles
    B: qpT[e,i]  = sum_d wqT[d,e] qT[d,i]            (128 MMs)
    C: sT[j,i]   = sum_e kpT[e,j] qpT[e,i]           (256 MMs)
       eT[j,i]   = exp(sT/32)  (ACT, fused scale, stays in SBUF)
       colsum[i] = sum_j eT[j,i]: partial sums over j on the vector
       engine, then one gpsimd partition_all_reduce per query slice
       (frees the PE of 32 ones-matmuls and 2 PSUM banks -> psmm
       rotates 8 banks)
    F: outT[e,i] = (sum_j vp[j,e] eT[j,i]) * (1/colsum[i])  (256 MMs)

DMA-descriptor generation is the hidden serial resource: each
dma_start costs ~0.6us of descriptor generation on its issuing
engine's sequencer, strictly in program order. All six inputs are
host-rearranged to (128, 8192) partition-major layouts (16KB rows),
so stream granularity is a free choice: critical first-chain streams
(k, wk, q) go as 8x[128,1024] descs (data streams in d-pair by
d-pair; a single big desc would arrive all-or-nothing ~15us late),
bulk streams (wv, vs, wq) as 2x[128,4096] descs. Input desc-gen
drops from ~30us (48 descs, d-tiled) to ~16us (30 descs), pulling
the input-transfer completion -- which bounds the kp collective's
all-lane DMA fence -- several us earlier. Ring assignment:
  sync ring  (no compute): wk, wv, vs, wq, q, then the collective
      readbacks -- it can safely stall on the collective-done sems.
      kp readback is front-loaded (a [128,256] quarter wave, split
      4 sync + 4 scalar so j0-1 unblock ~2.5us after the collective
      lands) then [128,768] remainders, then hr=1 full rows; the
      hr=1 transfers complete before the vp collective's HBM window
      opens (moving them later starves C's j8-15 by ~10us, measured).
  scalar ring (runs C's activations): k stream, A/E bounce writes,
      4 first-wave kp readback descs (safe: activations cannot be
      runnable before the same collective those descs wait on),
      F output stores. Other readbacks must NOT go here: their
      collective-done waits would queue C's activations behind them
      and deadlock the PSUM rotation (~44us stall, measured).
Both readbacks are emitted after phase B so the vp collective's
auto-generated DMA-lane fence is not dragged behind the kp readback
descriptors in the schedule.

Phase-F accumulators come from the same rotating PSUM pool as the
earlier phases (a fresh pool would wait on the colsum/reciprocal
chain before its first bank frees and stall the PE). The last output
tile runs as two 256-wide column chains so its mul+store overlaps
the final matmuls instead of trailing them.

918 matmuls/core; steady-state issue interval is the full-rate 216ns
(512 cols @ ~2.4GHz; LDWEIGHTS prefetch fully hidden). Measured
end-to-end relative error vs the fp32 reference: ~6e-3 (bf16
quantization). HW exec 218.9-227us depending on device DVFS state
and collective-firmware duration (22-38us observed); baseline 248us.
Remaining non-kernel costs: ~7.5us framework preamble, ~3.5us power
throttle, and the pair-AllGather latency floor.
"""

import numpy as np

_B, _S, _DK = 4, 2048, 1024
_HALF = _S // 2
_N_CORES = 8
_P = 128
_PAIRS = [[0, 1], [2, 3], [4, 5], [6, 7]]

_CACHE = {}


def _emit(tc, qT, kTh, vTh, wqT, wkT, wvT, outT, cc, DK, S, HALF, mm_dt):
    import concourse.bass as bass
    from concourse import bass_isa, mybir

    nc = tc.nc
    ts = bass.ts
    P = _P
    NF = 512
    KH = S // 2            # local key/value rows
    DT = DK // P           # contraction tiles (d)
    ET = DK // P           # output-feature tiles (e)
    JT = S // P            # global key tiles (j)
    ISL = HALF // NF       # query slices (i)
    JSLH = KH // NF        # local key slices
    ESL = DK // NF         # feature slices
    JGN = KH // NF         # local vT chunk groups
    JPG = NF // P          # j-tiles per vT chunk
    NORM = 1.0 / float(np.sqrt(DK))
    f32 = mybir.dt.float32
    AF = mybir.ActivationFunctionType
    kp_in, kp_out, vp_in, vp_out = cc

    _cms = {}

    def opn(**kw):
        cm = tc.tile_pool(**kw)
        pool = cm.__enter__()
        _cms[id(pool)] = cm
        return pool

    def cls(*pools):
        for pool in pools:
            _cms.pop(id(pool)).__exit__(None, None, None)

    # ---------------- pools ----------------
    # LEFT stack: misc | x (stream rotation) | kpT | vp | qpT
    # RIGHT stack: stage | cst (bounce staging) | weights | eT
    misc = opn(name="misc", bufs=1, side="left")
    xp = opn(name="xp", bufs=1, side="left")
    stage = opn(name="stage", bufs=4, side="right")
    cstp = opn(name="cstp", bufs=1, side="right")
    wp = opn(name="wp", bufs=1, side="right")
    psmm = opn(name="psmm", bufs=8, space="PSUM")

    ones = misc.tile([P, P], mm_dt, tag="ones")
    nc.vector.memset(ones[:], 1.0)
    recip = misc.tile([P, HALF], f32, tag="recip")
    cs_acc = [
        misc.tile([P, NF], f32, tag=f"csa{i}", name=f"csa{i}") for i in range(ISL)
    ]
    cs_red = [
        misc.tile([P, NF], f32, tag=f"csr{i}", name=f"csr{i}") for i in range(ISL)
    ]

    # input stream tiles are all dedicated (no slot rotation, no waits):
    # k and vs in the x pool as full [P, KH] rows, q in the spare wv slots.
    def cst_tile(nm):
        return cstp.tile([P, NF], mm_dt, tag="cst", bufs=12, name=nm)

    # big static tiles allocated up front so the k stream pool (dead
    # after A) is the top of the left stack and can be released early
    kp_pool = opn(name="kpp", bufs=1, side="left")
    kpT = [kp_pool.tile([P, S], mm_dt, tag=f"kp{e}", name=f"kp{e}") for e in range(ET)]
    vp_pool = opn(name="vpp", bufs=1, side="left")
    vp = [vp_pool.tile([P, DK], mm_dt, tag=f"vp{j}", name=f"vp{j}") for j in range(JT)]
    kxp = opn(name="kxp", bufs=1, side="left")

    # ---------------- PE warm-up while first DMAs land ----------------
    warm_ps = psmm.tile([P, P], f32, tag="mm", name="warm_ps")
    for _ in range(24):
        nc.tensor.matmul(warm_ps[:], ones[:], ones[:], start=True, stop=True)

    # All six inputs arrive host-rearranged into (128, 8192) partition-
    # major layouts (16KB per partition row), so each stream is one or
    # two large contiguous descriptors instead of 8-16: input desc-gen
    # drops from ~30us to ~6us, the first A chain's tiles arrive as one
    # block, and the input-transfer completion (which bounds the kp
    # collective's fence) moves ~8us earlier.
    #   k/q:  free order [half, d, 512]   (half = js / isl)
    #   wk:   free order [h, d, 512]      (h = e//4)
    #   wv/wq/vs: free order [d, 1024]
    EPC = NF // P
    HB = DT * NF  # 4096: one half-block of the [half, d, 512] layouts

    # critical first-chain streams (k, wk h0) go at [128,1024] grain so
    # data streams in d-pair by d-pair instead of one late block; bulk
    # streams (wv, vs, wq) stay as two big descs each.
    DP = 2 * NF
    k_all = kxp.tile([P, DT * KH], mm_dt, tag="k", name="k_all")
    for o in range(0, 2 * HB, DP):
        nc.scalar.dma_start(k_all[:, o : o + DP], kTh[:, o : o + DP])
    wk_all = wp.tile([P, DT * DK], mm_dt, tag="wk", name="wk_all")
    for o in range(0, 2 * HB, DP):
        nc.sync.dma_start(wk_all[:, o : o + DP], wkT[:, o : o + DP])
    wv_all = wp.tile([P, DT * DK], mm_dt, tag="wv", name="wv_all")
    nc.sync.dma_start(wv_all[:, 0:HB], wvT[:, 0:HB])
    nc.sync.dma_start(wv_all[:, HB : 2 * HB], wvT[:, HB : 2 * HB])
    vs_all = xp.tile([P, DT * KH], mm_dt, tag="vs", name="vs_all")
    nc.sync.dma_start(vs_all[:, 0:HB], vTh[:, 0:HB])
    nc.sync.dma_start(vs_all[:, HB : 2 * HB], vTh[:, HB : 2 * HB])

    def wk_slice(d, e):
        o = (e // EPC) * HB + d * NF + (e % EPC) * P
        return wk_all[:, o : o + P]

    # ---------------- phase A: local kpT half -> bounce -> AllGather ----
    for js in range(JSLH):
        for e in range(ET):
            ps = psmm.tile([P, NF], f32, tag="mm")
            for d in range(DT):
                nc.tensor.matmul(
                    ps[:],
                    wk_slice(d, e),
                    k_all[:, js * HB + d * NF : js * HB + (d + 1) * NF],
                    start=(d == 0),
                    stop=(d == DT - 1),
                )
            st = cst_tile(f"kpb{js}_{e}")
            nc.vector.tensor_copy(st[:], ps[:])
            nc.scalar.dma_start(kp_in[ts(e, P), ts(js, NF)], st[:])
    nc.gpsimd.collective_compute(
        "AllGather",
        mybir.AluOpType.bypass,
        replica_groups=_PAIRS,
        ins=[kp_in[:, :]],
        outs=[kp_out[:, :]],
    )

    # wq + q loads ride the sync ring behind the vs stream
    wq_all = wp.tile([P, DT * DK], mm_dt, tag="wq", name="wq_all")
    nc.sync.dma_start(wq_all[:, 0:HB], wqT[:, 0:HB])
    nc.sync.dma_start(wq_all[:, HB : 2 * HB], wqT[:, HB : 2 * HB])
    q_all = wp.tile([P, DT * DK], mm_dt, tag="q", name="q_all")
    for o in range(0, 2 * HB, DP):
        nc.sync.dma_start(q_all[:, o : o + DP], qT[:, o : o + DP])

    # ---------------- phase E: local vp half -> bounce -> AllGather ----
    for g in range(JGN):
        for jin in range(JPG):
            for es in range(ESL):
                ps = psmm.tile([P, NF], f32, tag="mm")
                for d in range(DT):
                    nc.tensor.matmul(
                        ps[:],
                        vs_all[
                            :,
                            d * DK + (g * JPG + jin) * P : d * DK
                            + (g * JPG + jin + 1) * P,
                        ],
                        wv_all[:, d * DK + es * NF : d * DK + (es + 1) * NF],
                        start=(d == 0),
                        stop=(d == DT - 1),
                    )
                st = cst_tile(f"vpb{g}_{jin}_{es}")
                nc.vector.tensor_copy(st[:], ps[:])
                nc.scalar.dma_start(
                    vp_in[ts(g * JPG + jin, P), ts(es, NF)], st[:]
                )
    nc.gpsimd.collective_compute(
        "AllGather",
        mybir.AluOpType.bypass,
        replica_groups=_PAIRS,
        ins=[vp_in[:, :]],
        outs=[vp_out[:, :]],
    )

    # ---------------- phase B: qpT = (q @ wq.T).T ----------------
    # k stream tiles are dead after A; free their SBUF so qpT fits
    cls(kxp)
    qp_pool = opn(name="qpp", bufs=1, side="left")
    qpT = [
        qp_pool.tile([P, HALF], mm_dt, tag=f"qp{e}", name=f"qp{e}") for e in range(ET)
    ]
    for isl in range(ISL):
        for e in range(ET):
            ps = psmm.tile([P, NF], f32, tag="mm")
            for d in range(DT):
                nc.tensor.matmul(
                    ps[:],
                    wq_all[:, d * DK + e * P : d * DK + (e + 1) * P],
                    q_all[:, isl * HB + d * NF : isl * HB + (d + 1) * NF],
                    start=(d == 0),
                    stop=(d == DT - 1),
                )
            nc.vector.tensor_copy(qpT[e][:, ts(isl, NF)], ps[:])

    # Collective readbacks, both emitted after B so the vp collective's
    # auto-generated DMA barrier isn't dragged behind them in the
    # schedule. Sync ring (no compute shares that sequencer; it can
    # safely stall on the collective-done semaphore). kp first: its
    # collective lands earlier and C consumes it first. kp is
    # front-loaded as a [128,256] quarter wave (unblocks j0-1 right
    # after the collective lands), then [128,768] remainders, then
    # hr=1 full rows; desc-gen then outpaces C's consumption.
    QW = 256
    for e in range(4):
        nc.sync.dma_start(kpT[e][:, 0:QW], kp_out[ts(e, P), 0:QW])
    for e in range(4, ET):
        nc.scalar.dma_start(kpT[e][:, 0:QW], kp_out[ts(e, P), 0:QW])
    for e in range(ET):
        nc.sync.dma_start(kpT[e][:, QW:KH], kp_out[ts(e, P), QW:KH])
    for e in range(ET):
        nc.sync.dma_start(kpT[e][:, KH : 2 * KH], kp_out[ts(ET + e, P), :])
    for hr in range(2):
        for jl in range(JT // 2):
            nc.sync.dma_start(
                vp[hr * (JT // 2) + jl][:, :], vp_out[ts(hr * (JT // 2) + jl, P), :]
            )

    # weights are dead after B; free their SBUF so eT can live there
    cls(wp)

    # ---------------- phase C: sT -> exp -> eT (SBUF) + trailing colsum ----
    et_pool = opn(name="etp", bufs=1, side="right")
    eT = [et_pool.tile([P, HALF], mm_dt, tag=f"et{j}", name=f"et{j}") for j in range(JT)]
    for j in range(JT):
        for isl in range(ISL):
            ps = psmm.tile([P, NF], f32, tag="mm")
            for e in range(ET):
                nc.tensor.matmul(
                    ps[:],
                    kpT[e][:, ts(j, P)],
                    qpT[e][:, ts(isl, NF)],
                    start=(e == 0),
                    stop=(e == ET - 1),
                )
            nc.scalar.activation(eT[j][:, ts(isl, NF)], ps[:], AF.Exp, scale=NORM)
            # colsum rides the vector engine (partial sums over j) plus a
            # gpsimd cross-partition reduce, freeing the PE of 32
            # ones-matmuls
            if j == 0:
                nc.vector.tensor_copy(cs_acc[isl][:], eT[0][:, ts(isl, NF)])
            else:
                nc.vector.tensor_tensor(
                    cs_acc[isl][:],
                    cs_acc[isl][:],
                    eT[j][:, ts(isl, NF)],
                    op=mybir.AluOpType.add,
                )
            if j == JT - 1:
                nc.gpsimd.partition_all_reduce(
                    cs_red[isl][:],
                    cs_acc[isl][:],
                    channels=P,
                    reduce_op=bass_isa.ReduceOp.add,
                )
                nc.vector.reciprocal(recip[:, ts(isl, NF)], cs_red[isl][:])

    # ---------------- phase F: outT = (eT.T @ vp).T * recip ----------------
    # Accumulators come from the same rotating psmm pool (a fresh PSUM pool
    # would wait on the colsum/reciprocal chain before its first bank frees).
    # The last (isl,e) tile runs as two 256-wide column chains so its
    # mul+store overlaps the final matmuls instead of trailing them.
    HW = NF // 2
    for isl in range(ISL):
        for e in range(ET):
            if isl == ISL - 1 and e == ET - 1:
                for half in range(2):
                    pfh = psmm.tile([P, NF], f32, tag="mm", name=f"pfl{half}")
                    for j in range(JT):
                        nc.tensor.matmul(
                            pfh[:, 0:HW],
                            vp[j][:, ts(e, P)],
                            eT[j][:, isl * NF + half * HW : isl * NF + (half + 1) * HW],
                            start=(j == 0),
                            stop=(j == JT - 1),
                        )
                    oth = stage.tile([P, HW], f32, tag="ostl", bufs=2)
                    nc.vector.tensor_mul(
                        oth[:],
                        pfh[:, 0:HW],
                        recip[:, isl * NF + half * HW : isl * NF + (half + 1) * HW],
                    )
                    nc.scalar.dma_start(
                        outT[
                            ts(e, P),
                            isl * NF + half * HW : isl * NF + (half + 1) * HW,
                        ],
                        oth[:],
                    )
                continue
            pft = psmm.tile([P, NF], f32, tag="mm", name=f"pf{e}_{isl}")
            for j in range(JT):
                nc.tensor.matmul(
                    pft[:],
                    vp[j][:, ts(e, P)],
                    eT[j][:, ts(isl, NF)],
                    start=(j == 0),
                    stop=(j == JT - 1),
                )
            ot = stage.tile([P, NF], f32, tag="ost")
            nc.vector.tensor_mul(ot[:], pft[:], recip[:, ts(isl, NF)])
            nc.scalar.dma_start(outT[ts(e, P), ts(isl, NF)], ot[:])
    cls(qp_pool, vp_pool, kp_pool, xp, misc)
    cls(et_pool, cstp, stage)
    cls(psmm)


def build_program(DK=_DK, S=_S, HALF=_HALF, mm_dtype="bfloat16"):
    """Build + compile the per-core Bass program. Returns the Bacc object."""
    import concourse.tile as tile
    from concourse import bacc, mybir

    f32 = mybir.dt.float32
    mm_dt = getattr(mybir.dt, mm_dtype)
    KH = S // 2
    NF = 512

    nc = bacc.Bacc(
        "TRN2",
        target_bir_lowering=False,
        debug=False,
        enable_asserts=False,
        num_devices=_N_CORES,
    )
    # all inputs host-rearranged to (128, 8192) partition-major layouts
    BIG = (DK // 128) * DK  # 8192
    qT = nc.dram_tensor("qt", (128, BIG), mm_dt, kind="ExternalInput").ap()
    kTh = nc.dram_tensor("kt", (128, BIG), mm_dt, kind="ExternalInput").ap()
    vTh = nc.dram_tensor("vt", (128, BIG), mm_dt, kind="ExternalInput").ap()
    wqT = nc.dram_tensor("wqt", (128, BIG), mm_dt, kind="ExternalInput").ap()
    wkT = nc.dram_tensor("wkt", (128, BIG), mm_dt, kind="ExternalInput").ap()
    wvT = nc.dram_tensor("wvt", (128, BIG), mm_dt, kind="ExternalInput").ap()
    outT = nc.dram_tensor("outt", (DK, HALF), f32, kind="ExternalOutput").ap()

    # HBM bounce buffers for the pair AllGathers (one per tensor: each
    # collective carries a ~25us firmware latency floor, so fewer is faster)
    kp_in = nc.dram_tensor("kp_in", (DK, KH), mm_dt, kind="Internal").ap()
    kp_out = nc.dram_tensor("kp_out", (2 * DK, KH), mm_dt, kind="Internal").ap()
    vp_in = nc.dram_tensor("vp_in", (KH, DK), mm_dt, kind="Internal").ap()
    vp_out = nc.dram_tensor("vp_out", (2 * KH, DK), mm_dt, kind="Internal").ap()

    with tile.TileContext(nc) as tc:
        _emit(
            tc,
            qT,
            kTh,
            vTh,
            wqT,
            wkT,
            wvT,
            outT,
            (kp_in, kp_out, vp_in, vp_out),
            DK,
            S,
            HALF,
            mm_dt,
        )
    nc.compile()
    return nc


def _half_major(aT):
    """(1024, 1024) -> (128, 8192), free order [half, d, 512]."""
    return np.ascontiguousarray(
        aT.reshape(8, 128, 2, 512).transpose(1, 2, 0, 3).reshape(128, 8192)
    )


def _d_major(aT):
    """(1024, 1024) -> (128, 8192), free order [d, 1024]."""
    return np.ascontiguousarray(
        aT.reshape(8, 128, 1024).transpose(1, 0, 2).reshape(128, 8192)
    )


def _in_maps(q, k, v, wq, wk, wv):
    """Shard full inputs into 8 per-core input maps.

    Host-side transposes plus a partition-major rearrange to (128, 8192)
    so every device-side input stream is 1-2 large contiguous DMA
    descriptors (16KB per partition row)."""
    import ml_dtypes

    bf16 = ml_dtypes.bfloat16
    wqT = _d_major(np.ascontiguousarray(wq.T).astype(bf16))
    wkT = _half_major(np.ascontiguousarray(wk.T).astype(bf16))
    wvT = _d_major(np.ascontiguousarray(wv.T).astype(bf16))
    maps = []
    for c in range(_N_CORES):
        b, h = divmod(c, 2)
        sl = slice(h * _HALF, (h + 1) * _HALF)
        maps.append(
            {
                "qt": _half_major(np.ascontiguousarray(q[b, sl, :].T).astype(bf16)),
                "kt": _half_major(np.ascontiguousarray(k[b, sl, :].T).astype(bf16)),
                "vt": _d_major(np.ascontiguousarray(v[b, sl, :].T).astype(bf16)),
                "wqt": wqT,
                "wkt": wkT,
                "wvt": wvT,
            }
        )
    return maps


def kernel(q, k, v, wq, wk, wv):
    from concourse.bass_utils import run_bass_kernel_spmd

    q = np.asarray(q, np.float32)
    k = np.asarray(k, np.float32)
    v = np.asarray(v, np.float32)
    wq = np.asarray(wq, np.float32)
    wk = np.asarray(wk, np.float32)
    wv = np.asarray(wv, np.float32)

    if "nc" not in _CACHE:
        _CACHE["nc"] = build_program()
    nc = _CACHE["nc"]

    res = run_bass_kernel_spmd(
        nc, _in_maps(q, k, v, wq, wk, wv), core_ids=list(range(_N_CORES))
    )

    out = np.empty((_B, _S, _DK), np.float32)
    for c in range(_N_CORES):
        b, h = divmod(c, 2)
        out[b, h * _HALF : (h + 1) * _HALF, :] = res.results[c]["outt"].T
    return out
